# revision 1
# baseline (speedup 1.0000x reference)
"""Trainium2 Bass kernel for an encoder layer (LN -> MHA+bias/mask -> LN -> FFN).

Strategy: pure data parallelism. B=8 batch elements across 8 NeuronCores, one
element per core, weights replicated, no collectives.

Per-core dataflow (S=1024, H=512, NH=8, DH=64, FFN=2048, P=128):
  - x loaded as [128, 8, 512] (seq on partitions).
  - LN1 stats along free dim; y = (x-mu)*rstd (gamma/beta folded into weights
    on the host); yT built with PE transposes (needed as the contraction-side
    operand of every projection matmul).
  - qT/kT = W.T @ yT in [head_dim, seq] layout; v in [seq, head_dim] layout
    with a ones column appended per head (v_aug) so the PV matmul also
    produces softmax denominators.
  - scores computed transposed: sT[k,q] = kT.T @ qT per head, two heads
    row-packed into the 128-wide PE array (K=64 each).
  - e = exp(sT) * embT where embT = exp(biasT + (maskT-1)*1e9) is computed
    once on-device from host-pretransposed bias/mask. Masked entries underflow
    to exactly 0, so no -1e9 clamp or max-subtraction pass is needed.
  - oT_aug[65, q] = v_aug.T @ e accumulated over k tiles: rows 0-63 are the
    unnormalized context, row 64 is the softmax denominator. Normalization:
    r = 1/denom (DVE), broadcast via a K=1 outer-product matmul, multiply.
  - attn_out = oT.T @ Wo + x (residual), LN2, FFN with hT = gelu(W1.T @ y2T)
    kept transposed so FFN2 needs no transposes either.

All big matmuls use float32r operands (full PE rate at N=512, near-fp32
accuracy). hT/W2 optionally bf16 to fit SBUF.
"""

import os

os.environ.setdefault("MYCRO_LOCAL_CACHE", "1")

import sys

for _p in ("/opt/trn_rl_repo", "/root/.axon_site/_ro/trn_rl_repo"):
    if os.path.isdir(_p) and _p not in sys.path:
        sys.path.insert(0, _p)

from contextlib import ExitStack

import numpy as np

import concourse.bass as bass
import concourse.tile as tile
from concourse import bacc, mybir
from concourse.masks import make_identity

F32 = mybir.dt.float32
F32R = mybir.dt.float32r
BF16 = mybir.dt.bfloat16
I32 = mybir.dt.int32
AF = mybir.ActivationFunctionType
ALU = mybir.AluOpType

S = 1024
H = 512
NH = 8
DH = 64
FFN = 2048
P = 128
B = 8
NEG = -1e9
EPS = 1e-5
SSC = S // P     # 8 seq tiles of 128
CC = H // P      # 4 channel chunks
FT = FFN // P    # 16 ffn chunks
QC = S // 512    # 2 query chunks of 512

# hT / W2 dtype (bf16 halves SBUF; h is post-gelu so precision impact is small)
H_DT = BF16
# matmul-operand dtype: float32r = fp32 bits, full PE rate at N>=512.
# The BIR verifier requires producers of fp32r matmul operands to emit
# fp32r, so these tensors are declared fp32r end-to-end.
MM_DT = F32R


def build_program(stop_after=None):
    nc = bacc.Bacc(
        "TRN2",
        target_bir_lowering=False,
        debug=False,
        enable_asserts=False,
        num_devices=B,
    )

    dram = {}

    def din(name, shape, dt):
        dram[name] = nc.dram_tensor(name, shape, dt, kind="ExternalInput").ap()
        return dram[name]

    x_d = din("x", [S, H], F32)
    abT_d = din("abT", [S, S], F32)       # attn_bias transposed: [k, q]
    gmT_d = din("gmT", [S, S], I32)       # graph_mask transposed: [k, q]
    wq_d = din("wq", [H, H], MM_DT)         # diag(ln1_g) @ Wq * scale
    wk_d = din("wk", [H, H], MM_DT)         # diag(ln1_g) @ Wk
    wv_d = din("wv", [H, H], MM_DT)         # diag(ln1_g) @ Wv
    wo_d = din("wo", [H, H], MM_DT)
    w1_d = din("w1", [H, FFN], MM_DT)       # diag(ln2_g) @ W1
    w2_d = din("w2", [FFN, H], F32 if H_DT == F32 else BF16)
    bq_d = din("bq_pc", [P, CC], F32)     # (ln1_b@Wq+bq)*scale, partition-major
    bk_d = din("bk_pc", [P, CC], F32)
    b1_d = din("b1_pc", [P, FT], F32)     # ln2_b@W1+b1, partition-major
    bv_d = din("bv_bc", [P, H], F32)      # ln1_b@Wv+bv broadcast over partitions
    bo_d = din("bo_bc", [P, H], F32)
    b2_d = din("b2_bc", [P, H], F32)

    out_d = nc.dram_tensor("out", [S, H], F32, kind="ExternalOutput").ap()

    def _emit(tc, ctx):
        pool = ctx.enter_context(tc.tile_pool(name="main", bufs=1))
        stream = ctx.enter_context(tc.tile_pool(name="stream", bufs=2))
        spool = ctx.enter_context(tc.tile_pool(name="small", bufs=4))
        # PSUM: 2+2+2+2 slots = 8 banks exactly
        ps_mm = ctx.enter_context(tc.tile_pool(name="ps_mm", bufs=2, space="PSUM"))
        ps_s = ctx.enter_context(tc.tile_pool(name="ps_s", bufs=2, space="PSUM"))
        ps_o = ctx.enter_context(tc.tile_pool(name="ps_o", bufs=2, space="PSUM"))
        ps_sm = ctx.enter_context(tc.tile_pool(name="ps_sm", bufs=2, space="PSUM"))

        def dump_and_stop(srcs):
            # debug: copy arbitrary 512-element-per-partition views to out rows
            for i, ap in enumerate(srcs[:SSC]):
                dt_ = stream.tile([P, H], F32, tag="dump")
                dst = dt_[:]
                if len(ap.shape) == 3:
                    dst = dst.rearrange(
                        "p (a b) -> p a b", a=ap.shape[1], b=ap.shape[2]
                    )
                nc.vector.tensor_copy(dst, ap)
                nc.sync.dma_start(out_d[i * P:(i + 1) * P], dt_[:])

        # ---- persistent SBUF tensors ----
        ident = pool.tile([P, P], F32, tag="ident")
        make_identity(nc, ident[:])
        x_sb = pool.tile([P, SSC, H], F32, tag="x")        # becomes x2 in place
        embT = pool.tile([P, SSC, S], F32, tag="big4mb")  # [k_in, kt, q]
        yT = pool.tile([P, CC, S], MM_DT, tag="yT")          # [c_in, cc, s]
        v_aug = pool.tile([P, SSC, NH, DH + 1], MM_DT, tag="vaug")
        oT = pool.tile([P, CC, S], MM_DT, tag="oT")          # [c_in, cc, s]

        wq_sb = pool.tile([P, CC, H], MM_DT, tag="wslot0")
        wk_sb = pool.tile([P, CC, H], MM_DT, tag="wslot1")
        wv_sb = pool.tile([P, CC, H], MM_DT, tag="wslot2")
        wo_sb = pool.tile([P, CC, H], MM_DT, tag="wslot3")
        bq_sb = pool.tile([P, CC], F32, tag="bq")
        bk_sb = pool.tile([P, CC], F32, tag="bk")
        b1_sb = pool.tile([P, FT], F32, tag="b1")
        bv_sb = pool.tile([P, H], F32, tag="bv")
        bo_sb = pool.tile([P, H], F32, tag="bo")
        b2_sb = pool.tile([P, H], F32, tag="b2")

        for i in range(CC):
            nc.sync.dma_start(wq_sb[:, i], wq_d[i * P:(i + 1) * P])
            nc.sync.dma_start(wk_sb[:, i], wk_d[i * P:(i + 1) * P])
            nc.sync.dma_start(wv_sb[:, i], wv_d[i * P:(i + 1) * P])
            nc.sync.dma_start(wo_sb[:, i], wo_d[i * P:(i + 1) * P])
        nc.sync.dma_start(bq_sb[:], bq_d)
        nc.sync.dma_start(bk_sb[:], bk_d)
        nc.sync.dma_start(b1_sb[:], b1_d)
        nc.sync.dma_start(bv_sb[:], bv_d)
        nc.sync.dma_start(bo_sb[:], bo_d)
        nc.sync.dma_start(b2_sb[:], b2_d)
        for i in range(SSC):
            nc.sync.dma_start(x_sb[:, i], x_d[i * P:(i + 1) * P])

        # ones columns of v_aug (DVE copy from an fp32 ones tile; strided
        # memset on an fp32r tile fails walrus ISA checks)
        ones_col = pool.tile([P, 1], F32, tag="ones_col")
        nc.gpsimd.memset(ones_col[:], 1.0)
        nc.vector.tensor_copy(
            v_aug[:, :, :, DH:DH + 1],
            ones_col[:].to_broadcast((P, SSC, NH, 1)),
        )

        # ---- embT = exp(biasT + (maskT-1)*1e9), computed per k-tile ----
        with tc.tile_pool(name="embp", bufs=2) as embp:
            for kt in range(SSC):
                bt = embp.tile([P, S], F32, tag="emb_b")
                mt = embp.tile([P, S], I32, tag="emb_m")
                nc.sync.dma_start(bt[:], abT_d[kt * P:(kt + 1) * P])
                nc.sync.dma_start(mt[:], gmT_d[kt * P:(kt + 1) * P])
                mb = embp.tile([P, S], F32, tag="emb_f")
                # mask*1e9 - 1e9 -> 0 (keep) or -1e9 (drop)
                nc.vector.tensor_scalar(mb[:], mt[:], 1e9, -1e9, ALU.mult, ALU.add)
                nc.vector.tensor_tensor(mb[:], mb[:], bt[:], ALU.add)
                nc.scalar.activation(embT[:, kt], mb[:], AF.Exp)

        # ---- LN helper ----
        def layer_norm(src_tile, y_out):
            """y_out[:] = (src - mean)/sqrt(var+eps), stats along free dim."""
            sumsq = spool.tile([P, 1], F32, tag="sumsq")
            sumx = spool.tile([P, 1], F32, tag="sumx")
            xsq = stream.tile([P, H], F32, tag="xsq")
            nc.vector.tensor_tensor(xsq[:], src_tile, src_tile, ALU.mult)
            nc.vector.reduce_sum(sumsq[:], xsq[:], axis=mybir.AxisListType.X)
            nc.vector.reduce_sum(sumx[:], src_tile, axis=mybir.AxisListType.X)
            mean = spool.tile([P, 1], F32, tag="mean")
            nc.vector.tensor_scalar_mul(mean[:], sumx[:], 1.0 / H)
            ex2 = spool.tile([P, 1], F32, tag="ex2")
            nc.vector.tensor_scalar_mul(ex2[:], sumsq[:], 1.0 / H)
            msq = spool.tile([P, 1], F32, tag="msq")
            nc.vector.tensor_tensor(msq[:], mean[:], mean[:], ALU.mult)
            veps = spool.tile([P, 1], F32, tag="veps")
            nc.vector.tensor_tensor(veps[:], ex2[:], msq[:], ALU.subtract)
            nc.vector.tensor_scalar_add(veps[:], veps[:], EPS)
            lnv = spool.tile([P, 1], F32, tag="lnv")
            nc.scalar.activation(lnv[:], veps[:], AF.Ln)
            rstd = spool.tile([P, 1], F32, tag="rstd")
            # rstd = exp(-0.5*ln(var+eps)); keeps ACT in the exp/ln table set
            nc.scalar.activation(rstd[:], lnv[:], AF.Exp, scale=-0.5)
            nmr = spool.tile([P, 1], F32, tag="nmr")
            nc.vector.tensor_tensor(nmr[:], mean[:], rstd[:], ALU.mult)
            nc.vector.tensor_scalar_mul(nmr[:], nmr[:], -1.0)
            nc.scalar.activation(y_out, src_tile, AF.Identity, bias=nmr[:], scale=rstd[:])

        def transpose_into(y_tile, dst, sc):
            """PE-transpose y_tile [128, H] into dst [P, CC, S] at seq block sc."""
            for cb in range(CC):
                pst = ps_sm.tile([P, 512], F32, tag="ps_small")
                nc.tensor.transpose(
                    pst[:, 0:P], y_tile[:, cb * P:(cb + 1) * P], ident[:]
                )
                if cb % 2 == 0:
                    nc.scalar.copy(dst[:, cb, sc * P:(sc + 1) * P], pst[:, 0:P])
                else:
                    nc.vector.tensor_copy(dst[:, cb, sc * P:(sc + 1) * P], pst[:, 0:P])

        # ---- LN1 + yT ----
        for sc in range(SSC):
            y_t = stream.tile([P, H], F32, tag="y")
            layer_norm(x_sb[:, sc], y_t[:])
            transpose_into(y_t, yT, sc)

        if stop_after == "ln1":
            dump_and_stop([yT[:, i % CC, (i // CC) * 512:(i // CC) * 512 + H] for i in range(SSC)])
            return

        # ---- V projection: v[s, h] = yT.T @ Wv, into v_aug slots ----
        for sc in range(SSC):
            psv = ps_mm.tile([P, H], F32, tag="mm")
            for ci in range(CC):
                nc.tensor.matmul(
                    psv[:],
                    yT[:, ci, sc * P:(sc + 1) * P],
                    wv_sb[:, ci],
                    start=(ci == 0),
                    stop=(ci == CC - 1),
                )
            # add bias and scatter per head into v_aug[:, sc, h, 0:64]
            nc.vector.tensor_tensor(
                v_aug[:, sc, :, 0:DH],
                psv[:].rearrange("p (h d) -> p h d", h=NH),
                bv_sb[:].rearrange("p (h d) -> p h d", h=NH),
                ALU.add,
            )

        if stop_after == "qkv":
            dump_and_stop([v_aug[:, i, :, 0:DH] for i in range(SSC)])
            return

        # ---- per head-pair: qT/kT projection then attention ----
        with tc.tile_pool(name="attnp", bufs=2) as attnp, \
             tc.tile_pool(name="epool", bufs=4) as epool:
            for cc in range(CC):
                h0, h1 = 2 * cc, 2 * cc + 1
                qT_c = attnp.tile([P, S], MM_DT, tag="qT")
                kT_c = attnp.tile([P, S], MM_DT, tag="kT")
                for qc in range(QC):
                    psq = ps_mm.tile([P, 512], F32, tag="mm")
                    for ci in range(CC):
                        nc.tensor.matmul(
                            psq[:],
                            wq_sb[:, ci, cc * P:(cc + 1) * P],
                            yT[:, ci, qc * 512:(qc + 1) * 512],
                            start=(ci == 0),
                            stop=(ci == CC - 1),
                        )
                    nc.scalar.activation(
                        qT_c[:, qc * 512:(qc + 1) * 512], psq[:], AF.Identity,
                        bias=bq_sb[:, cc:cc + 1],
                    )
                    psk = ps_mm.tile([P, 512], F32, tag="mm")
                    for ci in range(CC):
                        nc.tensor.matmul(
                            psk[:],
                            wk_sb[:, ci, cc * P:(cc + 1) * P],
                            yT[:, ci, qc * 512:(qc + 1) * 512],
                            start=(ci == 0),
                            stop=(ci == CC - 1),
                        )
                    nc.scalar.activation(
                        kT_c[:, qc * 512:(qc + 1) * 512], psk[:], AF.Identity,
                        bias=bk_sb[:, cc:cc + 1],
                    )

                for qc in range(QC):
                    qs = slice(qc * 512, (qc + 1) * 512)
                    # both heads' scores row-packed (K=64 at row groups 0-1 /
                    # 2-3) run concurrently in the PE array; their PV
                    # accumulation groups live in separate PSUM banks.
                    pso0 = ps_o.tile([DH + 1, 512], F32, tag="o", name=f"pso0_{qc}")
                    pso1 = ps_o.tile([DH + 1, 512], F32, tag="o", name=f"pso1_{qc}")
                    for kt in range(SSC):
                        ks = slice(kt * P, (kt + 1) * P)
                        pss0 = ps_s.tile([P, 512], F32, tag="s", name=f"pss0_{qc}_{kt}")
                        pss1 = ps_s.tile([P, 512], F32, tag="s", name=f"pss1_{qc}_{kt}")
                        nc.tensor.matmul(
                            pss0[:], kT_c[0:DH, ks], qT_c[0:DH, qs],
                            tile_position=(0, 0),
                        )
                        nc.tensor.matmul(
                            pss1[:], kT_c[DH:P, ks], qT_c[DH:P, qs],
                            tile_position=(DH, 0),
                        )
                        for h, pss, pso in ((h0, pss0, pso0), (h1, pss1, pso1)):
                            et = epool.tile([P, 512], MM_DT, tag="e", name=f"et_{h}_{qc}_{kt}")
                            nc.scalar.activation(et[:], pss[:], AF.Exp)
                            et2 = epool.tile([P, 512], MM_DT, tag="e2", name=f"et2_{h}_{qc}_{kt}")
                            nc.vector.tensor_tensor(
                                et2[:], et[:], embT[:, kt, qs], ALU.mult
                            )
                            nc.tensor.matmul(
                                pso[:],
                                v_aug[:, kt, h],
                                et2[:],
                                start=(kt == 0),
                                stop=(kt == SSC - 1),
                            )
                    for h, pso in ((h0, pso0), (h1, pso1)):
                        rows = slice(0, DH) if h == h0 else slice(DH, P)
                        dn_t = spool.tile([1, 512], F32, tag="dn")
                        nc.scalar.copy(dn_t[:], pso[DH:DH + 1, :])
                        r_t = spool.tile([1, 512], F32, tag="recip")
                        nc.vector.reciprocal(r_t[:], dn_t[:])
                        bc_t = spool.tile([DH, 512], F32, tag="bc")
                        nc.gpsimd.partition_broadcast(bc_t[:], r_t[:])
                        nc.vector.tensor_tensor(
                            oT[rows, cc, qs], pso[0:DH, :], bc_t[:], ALU.mult
                        )

        if stop_after == "attn":
            dump_and_stop([oT[:, i % CC, (i // CC) * 512:(i // CC) * 512 + H] for i in range(SSC)])
            return

        # ---- output projection + residual (x2 overwrites x in place) ----
        for sc in range(SSC):
            pso = ps_mm.tile([P, H], F32, tag="mm")
            for ci in range(CC):
                nc.tensor.matmul(
                    pso[:],
                    oT[:, ci, sc * P:(sc + 1) * P],
                    wo_sb[:, ci],
                    start=(ci == 0),
                    stop=(ci == CC - 1),
                )
            nc.vector.tensor_tensor(x_sb[:, sc], pso[:], x_sb[:, sc], ALU.add)
            nc.gpsimd.tensor_tensor(x_sb[:, sc], x_sb[:, sc], bo_sb[:], ALU.add)

        if stop_after == "wo":
            dump_and_stop([x_sb[:, i] for i in range(SSC)])
            return

        # ---- LN2 + y2T (reuses the yT slot) ----
        y2T = pool.tile([P, CC, S], MM_DT, tag="yT")
        for sc in range(SSC):
            y_t = stream.tile([P, H], F32, tag="y")
            layer_norm(x_sb[:, sc], y_t[:])
            transpose_into(y_t, y2T, sc)

        if stop_after == "ln2":
            dump_and_stop([y2T[:, i % CC, (i // CC) * 512:(i // CC) * 512 + H] for i in range(SSC)])
            return

        # ---- FFN ----
        # W1 chunks reuse the four attention weight slots; W2 reuses v_aug's.
        w1_c = [
            pool.tile([P, FFN], MM_DT, tag=f"wslot{i}", name=f"w1_c{i}")
            for i in range(CC)
        ]
        for i in range(CC):
            nc.sync.dma_start(w1_c[i][:], w1_d[i * P:(i + 1) * P])
        w2_sb = pool.tile([P, FT, H], H_DT, tag="vaug")
        for i in range(FT):
            nc.sync.dma_start(w2_sb[:, i], w2_d[i * P:(i + 1) * P])
        hT = pool.tile([P, FT, S], H_DT, tag="big4mb")

        for ft in range(FT):
            for qc in range(QC):
                psh = ps_mm.tile([P, 512], F32, tag="mm")
                for ci in range(CC):
                    nc.tensor.matmul(
                        psh[:],
                        w1_c[ci][:, ft * P:(ft + 1) * P],
                        y2T[:, ci, qc * 512:(qc + 1) * 512],
                        start=(ci == 0),
                        stop=(ci == CC - 1),
                    )
                nc.scalar.activation(
                    hT[:, ft, qc * 512:(qc + 1) * 512], psh[:], AF.Gelu,
                    bias=b1_sb[:, ft:ft + 1],
                )

        for sc in range(SSC):
            psf = ps_mm.tile([P, H], F32, tag="mm")
            for ft in range(FT):
                lhs = hT[:, ft, sc * P:(sc + 1) * P]
                rhs = w2_sb[:, ft]
                nc.tensor.matmul(
                    psf[:],
                    lhs,
                    rhs,
                    start=(ft == 0),
                    stop=(ft == FT - 1),
                )
            o_t = stream.tile([P, H], F32, tag="out_t")
            nc.vector.tensor_tensor(o_t[:], psf[:], x_sb[:, sc], ALU.add)
            nc.gpsimd.tensor_tensor(o_t[:], o_t[:], b2_sb[:], ALU.add)
            nc.sync.dma_start(out_d[sc * P:(sc + 1) * P], o_t[:])

    with tile.TileContext(nc) as tc, ExitStack() as ctx:
        _emit(tc, ctx)

    nc.compile()
    return nc


def prepare_in_maps(inputs):
    """Host-side prep: fold LN affine params + attention scale into weights,
    pre-transpose bias/mask per batch element, build per-core input dicts."""
    f = lambda a: np.asarray(a, np.float32)
    x = f(inputs["x"])
    ab = f(inputs["attn_bias"])
    gm = np.asarray(inputs["graph_mask"], np.int32)
    g1, b1l = f(inputs["ln1_g"]), f(inputs["ln1_b"])
    g2, b2l = f(inputs["ln2_g"]), f(inputs["ln2_b"])
    scale = DH ** -0.5

    wq = (g1[:, None] * f(inputs["Wq"])) * scale
    bq = (b1l @ f(inputs["Wq"]) + f(inputs["bq"])) * scale
    wk = g1[:, None] * f(inputs["Wk"])
    bk = b1l @ f(inputs["Wk"]) + f(inputs["bk"])
    wv = g1[:, None] * f(inputs["Wv"])
    bv = b1l @ f(inputs["Wv"]) + f(inputs["bv"])
    wo = f(inputs["Wo"])
    bo = f(inputs["bo"])
    w1 = g2[:, None] * f(inputs["W1"])
    b1 = b2l @ f(inputs["W1"]) + f(inputs["b1"])
    w2 = f(inputs["W2"])
    b2 = f(inputs["b2"])

    w2_cast = w2
    if H_DT == BF16:
        import ml_dtypes
        w2_cast = w2.astype(ml_dtypes.bfloat16)

    shared = {
        "wq": np.ascontiguousarray(wq),
        "wk": np.ascontiguousarray(wk),
        "wv": np.ascontiguousarray(wv),
        "wo": np.ascontiguousarray(wo),
        "w1": np.ascontiguousarray(w1),
        "w2": np.ascontiguousarray(w2_cast),
        "bq_pc": np.ascontiguousarray(bq.reshape(CC, P).T),
        "bk_pc": np.ascontiguousarray(bk.reshape(CC, P).T),
        "b1_pc": np.ascontiguousarray(b1.reshape(FT, P).T),
        "bv_bc": np.ascontiguousarray(np.tile(bv[None, :], (P, 1))),
        "bo_bc": np.ascontiguousarray(np.tile(bo[None, :], (P, 1))),
        "b2_bc": np.ascontiguousarray(np.tile(b2[None, :], (P, 1))),
    }
    in_maps = []
    for b in range(B):
        m = dict(shared)
        m["x"] = np.ascontiguousarray(x[b])
        m["abT"] = np.ascontiguousarray(ab[b].T)
        m["gmT"] = np.ascontiguousarray(gm[b].T)
        in_maps.append(m)
    return in_maps


_NC_CACHE = {}


def _get_nc():
    if "nc" not in _NC_CACHE:
        _NC_CACHE["nc"] = build_program()
    return _NC_CACHE["nc"]


def kernel(**inputs) -> np.ndarray:
    from concourse import bass_utils

    nc = _get_nc()
    in_maps = prepare_in_maps(inputs)
    res = bass_utils.run_bass_kernel_spmd(nc, in_maps, core_ids=list(range(B)))
    out = np.stack([np.asarray(res.results[b]["out"]) for b in range(B)], axis=0)
    return out.astype(np.float32)


if __name__ == "__main__":
    nc = build_program()
    print("build+compile OK:",
          sum(len(insts) for insts in getattr(nc, "engine_programs", {}).values())
          if hasattr(nc, "engine_programs") else "n/a")



# revision 4
# speedup vs baseline: 4.0928x; 4.0928x over previous
"""Trainium2 Bass kernel for an encoder layer (LN -> MHA+bias/mask -> LN -> FFN).

Strategy: pure data parallelism. B=8 batch elements across 8 NeuronCores, one
element per core. The metric is wall-clock per SPMD call over an axon tunnel
(~96 MB/s effective H2D), so the design minimizes shipped bytes per call:

  - x shipped fp16 [S,H] (1MB/core), converted to fp32 in SBUF (residual path
    stays fp32).
  - attn_bias+graph_mask shipped as one fp8e4 tensor emb8T [k,q] (1MB/core):
    host precomputes emb = exp(bias - rowmax)*mask*128 (softmax is invariant
    to per-row scaling, so the rowmax shift + x128 just centers the fp8
    dynamic range). Device multiplies it into exp(scores); masked entries are
    exactly 0 so no -1e9 clamp or max-subtraction pass is needed.
  - weights bf16, sharded 8x across cores (0.75MB/core instead of 6MB) and
    AllGathered on-device over NeuronLink into DRAM bounce buffers.
  - v bias folded into the output-projection bias on host (sum p = 1), so
    bo' = bv@Wo + bo; ln scales/biases folded into W/b as in the reference.
  - output fp16 (halves the donated zero buffer shipped in AND the result
    shipped back).

Per-core dataflow (S=1024, H=512, NH=8, DH=64, FFN=2048, P=128) is otherwise
the transposed-attention scheme: yT built with PE transposes; qT/kT = W.T@yT
per head-pair row-packed (K=64 x2) in the PE array; v_aug carries a ones
column so the PV matmul also produces softmax denominators; FFN keeps hT
transposed so no further transposes are needed. All matmul operands bf16
(full PE rate), PSUM accumulation fp32.
"""

import os

os.environ.setdefault("MYCRO_LOCAL_CACHE", "1")

import sys

for _p in ("/opt/trn_rl_repo", "/root/.axon_site/_ro/trn_rl_repo"):
    if os.path.isdir(_p) and _p not in sys.path:
        sys.path.insert(0, _p)

from contextlib import ExitStack

import numpy as np
import ml_dtypes

import concourse.bass as bass
import concourse.tile as tile
from concourse import bacc, mybir
from concourse.masks import make_identity

F32 = mybir.dt.float32
F16 = mybir.dt.float16
BF16 = mybir.dt.bfloat16
F8 = mybir.dt.float8e4
AF = mybir.ActivationFunctionType
ALU = mybir.AluOpType

S = 1024
H = 512
NH = 8
DH = 64
FFN = 2048
P = 128
B = 8
EPS = 1e-5
SSC = S // P     # 8 seq tiles of 128
CC = H // P      # 4 channel chunks
FT = FFN // P    # 16 ffn chunks
QC = S // 512    # 2 query chunks of 512

MM_DT = BF16     # matmul-operand dtype (full PE rate, fp32 PSUM accumulate)
EMB_SCALE = 128.0  # per-row softmax scale freedom used to center fp8 range

NP_BF16 = ml_dtypes.bfloat16
NP_F8 = ml_dtypes.float8_e4m3


def build_program(use_collectives=True):
    nc = bacc.Bacc(
        "TRN2",
        target_bir_lowering=False,
        debug=False,
        enable_asserts=False,
        num_devices=B,
    )

    x16_d = nc.dram_tensor("x16", [S, H], F16, kind="ExternalInput").ap()
    emb8_d = nc.dram_tensor("emb8T", [S, S], F8, kind="ExternalInput").ap()
    # weight shards (1/8 of rows each); gathered on device. Row layout:
    # wqkvo = [wq; wk; wv; wo] (each [H,H], ln1_g + attn scale folded in).
    qkvo_rows = 4 * H // (B if use_collectives else 1)
    w1_rows = H // (B if use_collectives else 1)
    w2_rows = FFN // (B if use_collectives else 1)
    wqkvo_d = nc.dram_tensor("wqkvo_s", [qkvo_rows, H], MM_DT, kind="ExternalInput").ap()
    w1_d = nc.dram_tensor("w1_s", [w1_rows, FFN], MM_DT, kind="ExternalInput").ap()
    w2_d = nc.dram_tensor("w2_s", [w2_rows, H], MM_DT, kind="ExternalInput").ap()
    ball_d = nc.dram_tensor("ball", [P, 24], F32, kind="ExternalInput").ap()   # bq|bk|b1
    bbc_d = nc.dram_tensor("bbc", [2, H], F32, kind="ExternalInput").ap()      # bo'|b2

    out_d = nc.dram_tensor("out", [S, H], F16, kind="ExternalOutput").ap()

    def _emit(tc, ctx):
        pool = ctx.enter_context(tc.tile_pool(name="main", bufs=1))
        stream = ctx.enter_context(tc.tile_pool(name="stream", bufs=2))
        spool = ctx.enter_context(tc.tile_pool(name="small", bufs=4))
        # PSUM: 2+2+2+2 slots = 8 banks exactly
        ps_mm = ctx.enter_context(tc.tile_pool(name="ps_mm", bufs=2, space="PSUM"))
        ps_s = ctx.enter_context(tc.tile_pool(name="ps_s", bufs=2, space="PSUM"))
        ps_o = ctx.enter_context(tc.tile_pool(name="ps_o", bufs=2, space="PSUM"))
        ps_sm = ctx.enter_context(tc.tile_pool(name="ps_sm", bufs=2, space="PSUM"))

        # ---- gather weight shards into full DRAM copies ----
        if use_collectives:
            dpool = ctx.enter_context(tc.tile_pool(name="dram", bufs=1, space="DRAM"))
            grp = [list(range(B))]
            gathered = []
            for src, rows, cols in (
                (wqkvo_d, 4 * H, H),
                (w1_d, H, FFN),
                (w2_d, FFN, H),
            ):
                bin_t = dpool.tile([rows // B, cols], MM_DT)
                g_t = dpool.tile([rows, cols], MM_DT)
                nc.gpsimd.dma_start(bin_t[:], src)
                nc.gpsimd.collective_compute(
                    "AllGather", ALU.bypass, replica_groups=grp,
                    ins=[bin_t.opt()], outs=[g_t.opt()],
                )
                gathered.append(g_t[:])
            wqkvo_g, w1_g, w2_g = gathered
        else:
            wqkvo_g, w1_g, w2_g = wqkvo_d, w1_d, w2_d

        # ---- persistent SBUF tensors ----
        ident = pool.tile([P, P], F32, tag="ident")
        make_identity(nc, ident[:])
        x_sb = pool.tile([P, SSC, H], F32, tag="x")        # becomes x2 in place
        embT = pool.tile([P, SSC, S], MM_DT, tag="big4mb")  # [k_in, kt, q]
        yT = pool.tile([P, CC, S], MM_DT, tag="yT")          # [c_in, cc, s]
        v_aug = pool.tile([P, SSC, NH, DH + 1], MM_DT, tag="vaug")
        oT = pool.tile([P, CC, S], MM_DT, tag="oT")          # [c_in, cc, s]

        wq_sb = pool.tile([P, CC, H], MM_DT, tag="wslot0")
        wk_sb = pool.tile([P, CC, H], MM_DT, tag="wslot1")
        wv_sb = pool.tile([P, CC, H], MM_DT, tag="wslot2")
        wo_sb = pool.tile([P, CC, H], MM_DT, tag="wslot3")
        b_all = pool.tile([P, 24], F32, tag="ball")        # bq 0:4 | bk 4:8 | b1 8:24
        bo_row = pool.tile([1, H], F32, tag="bo_row")
        b2_row = pool.tile([1, H], F32, tag="b2_row")
        bo_sb = pool.tile([P, H], F32, tag="bo")
        b2_sb = pool.tile([P, H], F32, tag="b2")

        for i in range(CC):
            nc.sync.dma_start(wq_sb[:, i], wqkvo_g[0 * H + i * P:0 * H + (i + 1) * P])
            nc.sync.dma_start(wk_sb[:, i], wqkvo_g[1 * H + i * P:1 * H + (i + 1) * P])
            nc.sync.dma_start(wv_sb[:, i], wqkvo_g[2 * H + i * P:2 * H + (i + 1) * P])
            nc.sync.dma_start(wo_sb[:, i], wqkvo_g[3 * H + i * P:3 * H + (i + 1) * P])
        nc.sync.dma_start(b_all[:], ball_d)
        nc.sync.dma_start(bo_row[:], bbc_d[0:1])
        nc.sync.dma_start(b2_row[:], bbc_d[1:2])
        nc.gpsimd.partition_broadcast(bo_sb[:], bo_row[:])
        nc.gpsimd.partition_broadcast(b2_sb[:], b2_row[:])

        # ---- load x (fp16 -> fp32) and emb (fp8 -> bf16) ----
        for sc in range(SSC):
            x16_t = stream.tile([P, H], F16, tag="x16")
            nc.sync.dma_start(x16_t[:], x16_d[sc * P:(sc + 1) * P])
            nc.vector.tensor_copy(x_sb[:, sc], x16_t[:])
        for kt in range(SSC):
            e8_t = stream.tile([P, S], F8, tag="e8")
            nc.sync.dma_start(e8_t[:], emb8_d[kt * P:(kt + 1) * P])
            nc.scalar.copy(embT[:, kt], e8_t[:])

        # ones columns of v_aug
        ones_col = pool.tile([P, 1], F32, tag="ones_col")
        nc.gpsimd.memset(ones_col[:], 1.0)
        nc.vector.tensor_copy(
            v_aug[:, :, :, DH:DH + 1],
            ones_col[:].to_broadcast((P, SSC, NH, 1)),
        )

        # ---- LN helper ----
        def layer_norm(src_tile, y_out):
            """y_out[:] = (src - mean)/sqrt(var+eps), stats along free dim."""
            sumsq = spool.tile([P, 1], F32, tag="sumsq")
            sumx = spool.tile([P, 1], F32, tag="sumx")
            xsq = stream.tile([P, H], F32, tag="xsq")
            nc.vector.tensor_tensor(xsq[:], src_tile, src_tile, ALU.mult)
            nc.vector.reduce_sum(sumsq[:], xsq[:], axis=mybir.AxisListType.X)
            nc.vector.reduce_sum(sumx[:], src_tile, axis=mybir.AxisListType.X)
            mean = spool.tile([P, 1], F32, tag="mean")
            nc.vector.tensor_scalar_mul(mean[:], sumx[:], 1.0 / H)
            ex2 = spool.tile([P, 1], F32, tag="ex2")
            nc.vector.tensor_scalar_mul(ex2[:], sumsq[:], 1.0 / H)
            msq = spool.tile([P, 1], F32, tag="msq")
            nc.vector.tensor_tensor(msq[:], mean[:], mean[:], ALU.mult)
            veps = spool.tile([P, 1], F32, tag="veps")
            nc.vector.tensor_tensor(veps[:], ex2[:], msq[:], ALU.subtract)
            nc.vector.tensor_scalar_add(veps[:], veps[:], EPS)
            lnv = spool.tile([P, 1], F32, tag="lnv")
            nc.scalar.activation(lnv[:], veps[:], AF.Ln)
            rstd = spool.tile([P, 1], F32, tag="rstd")
            # rstd = exp(-0.5*ln(var+eps)); keeps ACT in the exp/ln table set
            nc.scalar.activation(rstd[:], lnv[:], AF.Exp, scale=-0.5)
            nmr = spool.tile([P, 1], F32, tag="nmr")
            nc.vector.tensor_tensor(nmr[:], mean[:], rstd[:], ALU.mult)
            nc.vector.tensor_scalar_mul(nmr[:], nmr[:], -1.0)
            nc.scalar.activation(y_out, src_tile, AF.Identity, bias=nmr[:], scale=rstd[:])

        def transpose_into(y_tile, dst, sc):
            """PE-transpose y_tile [128, H] into dst [P, CC, S] at seq block sc."""
            for cb in range(CC):
                pst = ps_sm.tile([P, 512], F32, tag="ps_small")
                nc.tensor.transpose(
                    pst[:, 0:P], y_tile[:, cb * P:(cb + 1) * P], ident[:]
                )
                if cb % 2 == 0:
                    nc.scalar.copy(dst[:, cb, sc * P:(sc + 1) * P], pst[:, 0:P])
                else:
                    nc.vector.tensor_copy(dst[:, cb, sc * P:(sc + 1) * P], pst[:, 0:P])

        # ---- LN1 + yT ----
        for sc in range(SSC):
            y_t = stream.tile([P, H], F32, tag="y")
            layer_norm(x_sb[:, sc], y_t[:])
            transpose_into(y_t, yT, sc)

        # ---- V projection: v[s, h] = yT.T @ Wv, into v_aug slots ----
        for sc in range(SSC):
            psv = ps_mm.tile([P, H], F32, tag="mm")
            for ci in range(CC):
                nc.tensor.matmul(
                    psv[:],
                    yT[:, ci, sc * P:(sc + 1) * P],
                    wv_sb[:, ci],
                    start=(ci == 0),
                    stop=(ci == CC - 1),
                )
            # scatter per head into v_aug[:, sc, h, 0:64] (bv folded into bo')
            nc.vector.tensor_copy(
                v_aug[:, sc, :, 0:DH],
                psv[:].rearrange("p (h d) -> p h d", h=NH),
            )

        # ---- per head-pair: qT/kT projection then attention ----
        with tc.tile_pool(name="attnp", bufs=2) as attnp, \
             tc.tile_pool(name="epool", bufs=4) as epool:
            for cc in range(CC):
                h0, h1 = 2 * cc, 2 * cc + 1
                qT_c = attnp.tile([P, S], MM_DT, tag="qT")
                kT_c = attnp.tile([P, S], MM_DT, tag="kT")
                for qc in range(QC):
                    psq = ps_mm.tile([P, 512], F32, tag="mm")
                    for ci in range(CC):
                        nc.tensor.matmul(
                            psq[:],
                            wq_sb[:, ci, cc * P:(cc + 1) * P],
                            yT[:, ci, qc * 512:(qc + 1) * 512],
                            start=(ci == 0),
                            stop=(ci == CC - 1),
                        )
                    nc.scalar.activation(
                        qT_c[:, qc * 512:(qc + 1) * 512], psq[:], AF.Identity,
                        bias=b_all[:, cc:cc + 1],
                    )
                    psk = ps_mm.tile([P, 512], F32, tag="mm")
                    for ci in range(CC):
                        nc.tensor.matmul(
                            psk[:],
                            wk_sb[:, ci, cc * P:(cc + 1) * P],
                            yT[:, ci, qc * 512:(qc + 1) * 512],
                            start=(ci == 0),
                            stop=(ci == CC - 1),
                        )
                    nc.scalar.activation(
                        kT_c[:, qc * 512:(qc + 1) * 512], psk[:], AF.Identity,
                        bias=b_all[:, 4 + cc:5 + cc],
                    )

                for qc in range(QC):
                    qs = slice(qc * 512, (qc + 1) * 512)
                    # both heads' scores row-packed (K=64 at row groups 0-1 /
                    # 2-3) run concurrently in the PE array; their PV
                    # accumulation groups live in separate PSUM banks.
                    pso0 = ps_o.tile([DH + 1, 512], F32, tag="o", name=f"pso0_{qc}")
                    pso1 = ps_o.tile([DH + 1, 512], F32, tag="o", name=f"pso1_{qc}")
                    for kt in range(SSC):
                        ks = slice(kt * P, (kt + 1) * P)
                        pss0 = ps_s.tile([P, 512], F32, tag="s", name=f"pss0_{qc}_{kt}")
                        pss1 = ps_s.tile([P, 512], F32, tag="s", name=f"pss1_{qc}_{kt}")
                        nc.tensor.matmul(
                            pss0[:], kT_c[0:DH, ks], qT_c[0:DH, qs],
                            tile_position=(0, 0),
                        )
                        nc.tensor.matmul(
                            pss1[:], kT_c[DH:P, ks], qT_c[DH:P, qs],
                            tile_position=(DH, 0),
                        )
                        for h, pss, pso in ((h0, pss0, pso0), (h1, pss1, pso1)):
                            et = epool.tile([P, 512], MM_DT, tag="e", name=f"et_{h}_{qc}_{kt}")
                            nc.scalar.activation(et[:], pss[:], AF.Exp)
                            et2 = epool.tile([P, 512], MM_DT, tag="e2", name=f"et2_{h}_{qc}_{kt}")
                            nc.vector.tensor_tensor(
                                et2[:], et[:], embT[:, kt, qs], ALU.mult
                            )
                            nc.tensor.matmul(
                                pso[:],
                                v_aug[:, kt, h],
                                et2[:],
                                start=(kt == 0),
                                stop=(kt == SSC - 1),
                            )
                    for h, pso in ((h0, pso0), (h1, pso1)):
                        rows = slice(0, DH) if h == h0 else slice(DH, P)
                        dn_t = spool.tile([1, 512], F32, tag="dn")
                        nc.scalar.copy(dn_t[:], pso[DH:DH + 1, :])
                        r_t = spool.tile([1, 512], F32, tag="recip")
                        nc.vector.reciprocal(r_t[:], dn_t[:])
                        bc_t = spool.tile([DH, 512], F32, tag="bc")
                        nc.gpsimd.partition_broadcast(bc_t[:], r_t[:])
                        nc.vector.tensor_tensor(
                            oT[rows, cc, qs], pso[0:DH, :], bc_t[:], ALU.mult
                        )

        # ---- output projection + residual (x2 overwrites x in place) ----
        for sc in range(SSC):
            pso = ps_mm.tile([P, H], F32, tag="mm")
            for ci in range(CC):
                nc.tensor.matmul(
                    pso[:],
                    oT[:, ci, sc * P:(sc + 1) * P],
                    wo_sb[:, ci],
                    start=(ci == 0),
                    stop=(ci == CC - 1),
                )
            nc.vector.tensor_tensor(x_sb[:, sc], pso[:], x_sb[:, sc], ALU.add)
            nc.gpsimd.tensor_tensor(x_sb[:, sc], x_sb[:, sc], bo_sb[:], ALU.add)

        # ---- LN2 + y2T (reuses the yT slot) ----
        y2T = pool.tile([P, CC, S], MM_DT, tag="yT")
        for sc in range(SSC):
            y_t = stream.tile([P, H], F32, tag="y")
            layer_norm(x_sb[:, sc], y_t[:])
            transpose_into(y_t, y2T, sc)

        # ---- FFN ----
        # W1 chunks reuse the four attention weight slots; W2 reuses v_aug's.
        w1_c = [
            pool.tile([P, FFN], MM_DT, tag=f"wslot{i}", name=f"w1_c{i}")
            for i in range(CC)
        ]
        for i in range(CC):
            nc.sync.dma_start(w1_c[i][:], w1_g[i * P:(i + 1) * P])
        w2_sb = pool.tile([P, FT, H], MM_DT, tag="vaug")
        for i in range(FT):
            nc.sync.dma_start(w2_sb[:, i], w2_g[i * P:(i + 1) * P])
        hT = pool.tile([P, FT, S], MM_DT, tag="big4mb")

        for ft in range(FT):
            for qc in range(QC):
                psh = ps_mm.tile([P, 512], F32, tag="mm")
                for ci in range(CC):
                    nc.tensor.matmul(
                        psh[:],
                        w1_c[ci][:, ft * P:(ft + 1) * P],
                        y2T[:, ci, qc * 512:(qc + 1) * 512],
                        start=(ci == 0),
                        stop=(ci == CC - 1),
                    )
                nc.scalar.activation(
                    hT[:, ft, qc * 512:(qc + 1) * 512], psh[:], AF.Gelu,
                    bias=b_all[:, 8 + ft:9 + ft],
                )

        for sc in range(SSC):
            psf = ps_mm.tile([P, H], F32, tag="mm")
            for ft in range(FT):
                nc.tensor.matmul(
                    psf[:],
                    hT[:, ft, sc * P:(sc + 1) * P],
                    w2_sb[:, ft],
                    start=(ft == 0),
                    stop=(ft == FT - 1),
                )
            o_t = stream.tile([P, H], F32, tag="out_t")
            nc.vector.tensor_tensor(o_t[:], psf[:], x_sb[:, sc], ALU.add)
            nc.gpsimd.tensor_tensor(o_t[:], o_t[:], b2_sb[:], ALU.add)
            o16 = stream.tile([P, H], F16, tag="out16")
            nc.scalar.copy(o16[:], o_t[:])
            nc.sync.dma_start(out_d[sc * P:(sc + 1) * P], o16[:])

    with tile.TileContext(nc) as tc, ExitStack() as ctx:
        _emit(tc, ctx)

    nc.compile()
    return nc


def fold_weights(inputs):
    """Host-side prep: fold LN affine params, attention scale, and the V bias
    into weights/biases. Returns bf16/f32 arrays shared by all cores."""
    f = lambda a: np.asarray(a, np.float32)
    g1, b1l = f(inputs["ln1_g"]), f(inputs["ln1_b"])
    g2, b2l = f(inputs["ln2_g"]), f(inputs["ln2_b"])
    Wq, Wk, Wv, Wo = f(inputs["Wq"]), f(inputs["Wk"]), f(inputs["Wv"]), f(inputs["Wo"])
    W1, W2 = f(inputs["W1"]), f(inputs["W2"])
    scale = DH ** -0.5

    wq = (g1[:, None] * Wq) * scale
    bq = (b1l @ Wq + f(inputs["bq"])) * scale
    wk = g1[:, None] * Wk
    bk = b1l @ Wk + f(inputs["bk"])
    wv = g1[:, None] * Wv
    bv = b1l @ Wv + f(inputs["bv"])
    bo_eff = bv @ Wo + f(inputs["bo"])   # sum_k p_k = 1 folds bv through Wo
    w1 = g2[:, None] * W1
    b1_eff = b2l @ W1 + f(inputs["b1"])
    b2_eff = f(inputs["b2"])

    wqkvo = np.concatenate([wq, wk, wv, Wo], axis=0).astype(NP_BF16)  # [4H, H]
    w1c = np.ascontiguousarray(w1.astype(NP_BF16))                   # [H, FFN]
    w2c = np.ascontiguousarray(W2.astype(NP_BF16))                   # [FFN, H]
    ball = np.zeros((P, 24), np.float32)
    ball[:, 0:4] = bq.reshape(CC, P).T
    ball[:, 4:8] = bk.reshape(CC, P).T
    ball[:, 8:24] = b1_eff.reshape(FT, P).T
    bbc = np.ascontiguousarray(np.stack([bo_eff, b2_eff]).astype(np.float32))
    return {"wqkvo": wqkvo, "w1": w1c, "w2": w2c, "ball": ball, "bbc": bbc}


def make_in_maps(inputs, use_collectives=True):
    """Build per-core input dicts: fp16 x, fp8 pre-exp'd bias/mask, weight
    shards (or full weights when use_collectives=False)."""
    w = fold_weights(inputs)
    x = np.asarray(inputs["x"], np.float32)
    ab = np.asarray(inputs["attn_bias"], np.float32)
    gm = np.asarray(inputs["graph_mask"]) != 0            # [B, q, k]

    # emb[q,k] = exp(ab - rowmax_q) * mask * EMB_SCALE, in (0, EMB_SCALE].
    # Softmax over k is invariant to any per-q scale, so this only centers
    # the fp8 dynamic range (avoids both overflow and subnormal crush).
    masked = np.where(gm, ab, -np.inf)
    rowmax = masked.max(axis=-1, keepdims=True)           # [B, q, 1]
    rowmax = np.where(np.isfinite(rowmax), rowmax, 0.0)
    emb = np.exp(ab - rowmax) * gm * EMB_SCALE
    embT = np.swapaxes(emb, 1, 2)                         # [B, k, q]
    emb8 = embT.astype(NP_F8)

    in_maps = []
    for b in range(B):
        m = {
            "x16": np.ascontiguousarray(x[b].astype(np.float16)),
            "emb8T": np.ascontiguousarray(emb8[b]),
            "ball": w["ball"],
            "bbc": w["bbc"],
        }
        if use_collectives:
            qs, w1s, w2s = 4 * H // B, H // B, FFN // B
            m["wqkvo_s"] = np.ascontiguousarray(w["wqkvo"][b * qs:(b + 1) * qs])
            m["w1_s"] = np.ascontiguousarray(w["w1"][b * w1s:(b + 1) * w1s])
            m["w2_s"] = np.ascontiguousarray(w["w2"][b * w2s:(b + 1) * w2s])
        else:
            m["wqkvo_s"] = w["wqkvo"]
            m["w1_s"] = w["w1"]
            m["w2_s"] = w["w2"]
        in_maps.append(m)
    return in_maps


_NC_CACHE = {}


def _get_nc(use_collectives=True):
    key = ("nc", use_collectives)
    if key not in _NC_CACHE:
        _NC_CACHE[key] = build_program(use_collectives)
    return _NC_CACHE[key]


def kernel(**inputs) -> np.ndarray:
    from concourse import bass_utils

    nc = _get_nc()
    in_maps = make_in_maps(inputs)
    res = bass_utils.run_bass_kernel_spmd(nc, in_maps, core_ids=list(range(B)))
    out = np.stack([np.asarray(res.results[b]["out"]) for b in range(B)], axis=0)
    return out.astype(np.float32)


if __name__ == "__main__":
    nc = build_program()
    print("build+compile OK")
